# revision 21
# baseline (speedup 1.0000x reference)
"""Single-head self-attention (B=4, S=2048, D=1024) on 8 trn2 NeuronCores.

Sharding: core c -> (batch b = c//2, query half h = c%2); data-parallel over
batch, sequence-parallel over queries within a batch. Each core receives its
batch's x in both layouts (x^T d-major for the score projections, x native
t-major for the attention-weighted contraction) with its own seq-half first
(softmax is invariant to key permutation). The host gather is then a pure
concatenation of [1024, 1024] output blocks.

Per-core algorithm (no Q, K^T or V is ever materialized):
  G[d, s] = sum_d' Mkq[d, d'] xT[d', s] + ckq[d]     [1024, 1024]
            where Mkq = Wk @ Wq^T and ckq = Wk @ bq are fused on the host:
            scores[s, t] = q_s . k_t = x_t . (Mkq q_s + ckq) + const(s),
            and const(s) (the K-bias term) cancels in the softmax.
  scores^T[t, s] = sum_d xT[d, t] G[d, s]
  expP = exp(scores^T / 32); E = sum of expP tiles (DVE chain)
  l[s] via one N=2 matmul per query tile against a ones vector
  H^T[d, s] = sum_t x[t, d] expP[t, s]      (attn contracts x first)
  out[s, j] = (sum_d H^T[d, s] Wv[d, j]) / l[s] + bv[j]
The Q/K fusion drops the per-core matmul count to 776 N=512 instructions
(13.0 GFLOP/core, below the 15.05 unfused zero-duplication floor) with no
inter-core communication. All matmul operands are bf16 (fp32 PSUM
accumulate): measured relative error ~3e-3. bf16 also halves DMA/SBUF
traffic, so the fused weight, Wv and both x layouts are SBUF-resident for
the whole kernel — every input byte is pulled from HBM exactly once.
"""

import os
import sys
import types

import numpy as np

B, S, D = 4, 2048, 1024
HALF = S // 2  # 1024 queries per core
SCALE = 1.0 / 32.0  # 1/sqrt(D)
NC = 8
DC = D // 128  # 8 d-chunks
TT = S // 128  # 16 key tiles
SBLK = 512  # queries per s-block
NSB = HALF // SBLK  # 2 s-blocks

_CACHED_NC = None
LAST_RESULT = None  # BassKernelResults of the most recent run (for test.py)


def _ensure_axon_ntff_hook():
    """bass_utils' trace path needs antenv.axon_hooks; this image's antenv
    lacks it. Install a shim backed by trn_agent_boot's ctypes hook so
    BASS_TRACE=1 profiling works. No-op if already present/unavailable."""
    try:
        import antenv.axon_hooks  # noqa: F401

        return
    except ImportError:
        pass
    try:
        from trn_agent_boot.trn_boot import _ntff_profile_via_ctypes

        hook = _ntff_profile_via_ctypes("/opt/axon/libaxon_pjrt.so")
    except Exception:
        hook = None
    mod = types.ModuleType("antenv.axon_hooks")
    mod.get_axon_ntff_profile_hook = lambda: hook
    mod.set_axon_ntff_profile_hook = lambda h: None
    sys.modules["antenv.axon_hooks"] = mod


def build_kernel(tc, xt, xn, wkq, wv, ckq, bv, out):
    """Per-core attention with Q/K fused into one projection (see module
    docstring). All matmuls bf16 x bf16 -> fp32 PSUM. Every input is loaded
    exactly once into SBUF up front."""
    import concourse.bass as bass
    from concourse import mybir

    nc = tc.nc
    F32 = mybir.dt.float32
    F32R = mybir.dt.float32r
    BF16 = mybir.dt.bfloat16
    Identity = mybir.ActivationFunctionType.Identity
    Copy = mybir.ActivationFunctionType.Copy
    Exp = mybir.ActivationFunctionType.Exp

    xn_r = xn.rearrange("(tc p) d -> p tc d", p=128)  # [128, 16, 1024]
    out_r = out.rearrange("(su p) j -> su p j", p=128)  # [8, 128, 1024]

    with tc.tile_pool(name="persist", bufs=1) as persist:
        # -- persistent SBUF residents (per-partition bytes in parens) ------
        xT = persist.tile([128, DC, S], BF16)  # 32 KiB
        wkq_sb = persist.tile([128, DC, D], BF16)  # 16 KiB
        G = persist.tile([128, DC, HALF], BF16)  # 16 KiB
        ckq_sb = persist.tile([128, DC], F32)
        ones_f = persist.tile([128, 2], F32)
        ones_t = persist.tile([128, 2], F32R)

        nc.vector.memset(ones_f, 1.0)
        nc.vector.tensor_copy(ones_t, ones_f)

        # All issue queues fan out to the same 16 HW DMA engines (aggregate
        # ~358 GB/s), so the critical prefix (wkq + xT) owns ALL of them:
        # it is issued alone, in first-use order, on the scalar ring. The
        # Phase-B-only tensors (xn, wv, bv) are issued on the idle sync ring
        # from inside the Phase B pool, whose open-barrier naturally delays
        # them until Phase A's matmuls are done competing for HBM.
        nc.scalar.dma_start(wkq_sb, wkq)
        nc.scalar.dma_start(xT[:, :, 0:512], xt[0])
        nc.scalar.dma_start(xT[:, :, 512:1024], xt[1])
        nc.scalar.dma_start(ckq_sb, ckq)
        nc.scalar.dma_start(xT[:, :, 1024:1536], xt[2])
        nc.scalar.dma_start(xT[:, :, 1536:2048], xt[3])

        # ---- Phase A: G = Mkq @ x^T + ckq --------------------------------
        with tc.tile_pool(name="psa", bufs=2, space="PSUM") as psa:
            # PE warmup: tiny input-independent matmuls run during the input
            # DMA wait so the HAM clock gate is at 2.4 GHz when real work
            # arrives (it otherwise starts cold at 1.2 GHz).
            warm = psa.tile([2, 2], F32, tag="warm", bufs=1)
            for _ in range(80):
                nc.tensor.matmul(warm, ones_t, ones_t, start=True, stop=True)

            for sblk in range(NSB):
                for gc in range(DC):
                    gpsum = psa.tile([128, SBLK], F32, tag="gpsum")
                    for c in range(DC):
                        nc.tensor.matmul(
                            gpsum,
                            wkq_sb[:, c, gc * 128 : (gc + 1) * 128],
                            xT[:, c, sblk * SBLK : (sblk + 1) * SBLK],
                            start=(c == 0),
                            stop=(c == DC - 1),
                        )
                    nc.scalar.activation(
                        G[:, gc, sblk * SBLK : (sblk + 1) * SBLK],
                        gpsum,
                        Identity,
                        bias=ckq_sb[:, gc : gc + 1],
                    )

        # ---- Phase B: scores^T -> exp -> H^T -> out, per 512-query block -
        with (
            tc.tile_pool(name="pb", bufs=1) as pb,
            tc.tile_pool(name="psb", bufs=2, space="PSUM") as psb,
        ):
            xn_sb = pb.tile([128, TT, D], BF16, tag="xn")  # 32 KiB
            wv_sb = pb.tile([128, DC, D], BF16, tag="wv")  # 16 KiB
            bv_bc = pb.tile([128, D], F32, tag="bv")  # 4 KiB
            nc.sync.dma_start(xn_sb, xn_r)
            nc.sync.dma_start(wv_sb, wv)
            bv_bcast_ap = bass.AP(
                tensor=bv.tensor, offset=bv.offset, ap=[[0, 128]] + list(bv.ap)
            )
            nc.sync.dma_start(bv_bc, bv_bcast_ap)
            for sb in range(NSB):
                # scores^T + exp; E accumulates the softmax sums on DVE
                expP = pb.tile([128, TT, SBLK], BF16, tag="expP")
                E_t = pb.tile([128, SBLK], F32R, tag="E_t", bufs=1)
                for tt in range(TT):
                    spsum = psb.tile([128, SBLK], F32, tag="spsum")
                    for c in range(DC):
                        nc.tensor.matmul(
                            spsum,
                            xT[:, c, tt * 128 : (tt + 1) * 128],
                            G[:, c, sb * SBLK : (sb + 1) * SBLK],
                            start=(c == 0),
                            stop=(c == DC - 1),
                        )
                    nc.scalar.activation(expP[:, tt, :], spsum, Exp, scale=SCALE)
                    if tt == 1:
                        nc.vector.tensor_add(E_t, expP[:, 0, :], expP[:, 1, :])
                    elif tt > 1:
                        nc.vector.tensor_add(E_t, E_t, expP[:, tt, :])

                # H^T[d, s] = sum_t x[t, d] expP[t, s]
                H = pb.tile([128, DC, SBLK], BF16, tag="H")
                for dc in range(DC):
                    hpsum = psb.tile([128, SBLK], F32, tag="hpsum")
                    for tt in range(TT):
                        nc.tensor.matmul(
                            hpsum,
                            xn_sb[:, tt, dc * 128 : (dc + 1) * 128],
                            expP[:, tt, :],
                            start=(tt == 0),
                            stop=(tt == TT - 1),
                        )
                    nc.scalar.activation(H[:, dc, :], hpsum, Copy)

                # out[s, j] = (sum_d H^T[d, s] Wv[d, j]) / l[s] + bv[j]
                for su in range(SBLK // 128):
                    s0 = su * 128
                    lpsum = psb.tile([128, 2], F32, tag="lpsum", bufs=2)
                    nc.tensor.matmul(
                        lpsum, E_t[:, s0 : s0 + 128], ones_t, start=True, stop=True
                    )
                    recip = pb.tile([128, 1], F32, tag="recip", bufs=2)
                    nc.vector.reciprocal(recip, lpsum[:, 0:1])
                    o_sb = pb.tile([128, D], F32, tag="o_sb", bufs=2)
                    for jb in range(2):
                        opsum = psb.tile([128, 512], F32, tag="opsum")
                        for dc in range(DC):
                            nc.tensor.matmul(
                                opsum,
                                H[:, dc, s0 : s0 + 128],
                                wv_sb[:, dc, jb * 512 : (jb + 1) * 512],
                                start=(dc == 0),
                                stop=(dc == DC - 1),
                            )
                        oh = o_sb[:, jb * 512 : (jb + 1) * 512]
                        nc.vector.tensor_scalar_mul(oh, in0=opsum, scalar1=recip)
                        nc.vector.tensor_add(
                            oh, oh, bv_bc[:, jb * 512 : (jb + 1) * 512]
                        )
                    # one 512 KiB row-block per DMA on the idle sync HW ring
                    # (the gpsimd SWDGE ring is slow to drain the last block)
                    nc.sync.dma_start(out_r[sb * (SBLK // 128) + su], o_sb)


def build_nc():
    global _CACHED_NC
    if _CACHED_NC is not None:
        return _CACHED_NC
    import concourse.tile as tile
    from concourse import bacc, mybir

    F32 = mybir.dt.float32
    BF16 = mybir.dt.bfloat16
    nc = bacc.Bacc("TRN2", target_bir_lowering=False, debug=False)
    xt = nc.dram_tensor(
        "xt", [S // 512, 128, DC, 512], BF16, kind="ExternalInput"
    ).ap()
    xn = nc.dram_tensor("xn", [S, D], BF16, kind="ExternalInput").ap()
    wkq = nc.dram_tensor("wkq", [128, DC, D], BF16, kind="ExternalInput").ap()
    wv = nc.dram_tensor("wv", [128, DC, D], BF16, kind="ExternalInput").ap()
    ckq = nc.dram_tensor("ckq", [128, DC], F32, kind="ExternalInput").ap()
    bv = nc.dram_tensor("bv", [D], F32, kind="ExternalInput").ap()
    out = nc.dram_tensor("out", [HALF, D], F32, kind="ExternalOutput").ap()

    with tile.TileContext(nc) as tc:
        build_kernel(tc, xt, xn, wkq, wv, ckq, bv, out)
    nc.compile()
    _CACHED_NC = nc
    return nc


def _shard_inputs(x, Wq, bq, Wk, bk, Wv, bv):
    """Host-side prep: fused Q/K weight + per-core permuted x layouts."""
    import ml_dtypes

    BF = ml_dtypes.bfloat16
    # scores = q . k = x_t . (Mkq q_s + ckq): Mkq = Wk Wq^T, ckq = Wk bq.
    # The kernel consumes the weight in [d_in, d_out] = Mkq^T layout.
    Mkq = (Wk.astype(np.float64) @ Wq.astype(np.float64).T).astype(np.float32)
    ckq = (Wk.astype(np.float64) @ bq.astype(np.float64)).astype(np.float32)
    wkq_r = np.ascontiguousarray(
        Mkq.T.reshape(DC, 128, D).transpose(1, 0, 2).astype(BF)
    )
    wv_r = np.ascontiguousarray(Wv.reshape(DC, 128, D).transpose(1, 0, 2).astype(BF))
    ckq_r = np.ascontiguousarray(ckq.reshape(DC, 128).T)
    bv_c = np.ascontiguousarray(bv)

    in_maps = []
    for c in range(NC):
        b, h = divmod(c, 2)
        xb = x[b]
        if h:
            xb = np.concatenate([xb[HALF:], xb[:HALF]], axis=0)
        xb16 = xb.astype(BF)
        # x^T pre-tiled as [tb, p, c, u]: xt[tb, p, c, u] = x^T[c*128+p, tb*512+u]
        xt = np.ascontiguousarray(
            xb16.T.reshape(DC, 128, S // 512, 512).transpose(2, 1, 0, 3)
        )
        xn = np.ascontiguousarray(xb16)  # [S, D], same permutation
        in_maps.append(
            {
                "xt": xt,
                "xn": xn,
                "wkq": wkq_r,
                "wv": wv_r,
                "ckq": ckq_r,
                "bv": bv_c,
            }
        )
    return in_maps


def kernel(x, Wq, bq, Wk, bk, Wv, bv):
    global LAST_RESULT
    _ensure_axon_ntff_hook()
    from concourse import bass_utils

    x = np.asarray(x, dtype=np.float32)
    args = [np.asarray(a, dtype=np.float32) for a in (Wq, bq, Wk, bk, Wv, bv)]
    nc = build_nc()
    in_maps = _shard_inputs(x, *args)
    res = bass_utils.run_bass_kernel_spmd(nc, in_maps, core_ids=list(range(NC)))
    LAST_RESULT = res
    out = np.empty((B, S, D), dtype=np.float32)
    for c in range(NC):
        b, h = divmod(c, 2)
        out[b, h * HALF : (h + 1) * HALF, :] = res.results[c]["out"]
    return out


if __name__ == "__main__":
    rng = np.random.default_rng(0)
    init = 1.0 / 32.0
    x = rng.standard_normal((B, S, D), dtype=np.float32)
    mk = lambda *s: rng.uniform(-init, init, s).astype(np.float32)
    o = kernel(x, mk(D, D), mk(D), mk(D, D), mk(D), mk(D, D), mk(D))
    print("out", o.shape, o.dtype, float(np.abs(o).max()))


# revision 22
# speedup vs baseline: 1.0135x; 1.0135x over previous
"""Single-head self-attention (B=4, S=2048, D=1024) on 8 trn2 NeuronCores.

Sharding: core c -> (batch b = c//2, query half h = c%2); data-parallel over
batch, sequence-parallel over queries within a batch. Each core receives its
batch's x in both layouts (x^T d-major for the score projections, x native
t-major for the attention-weighted contraction) with its own seq-half first
(softmax is invariant to key permutation). The host gather is then a pure
concatenation of [1024, 1024] output blocks.

Per-core algorithm (no Q, K^T or V is ever materialized):
  G[d, s] = sum_d' Mkq[d, d'] xT[d', s] + ckq[d]     [1024, 1024]
            where Mkq = Wk @ Wq^T and ckq = Wk @ bq are fused on the host:
            scores[s, t] = q_s . k_t = x_t . (Mkq q_s + ckq) + const(s),
            and const(s) (the K-bias term) cancels in the softmax.
  scores^T[t, s] = sum_d xT[d, t] G[d, s]
  expP = exp(scores^T / 32); E = sum of expP tiles (DVE chain)
  l[s] via one N=2 matmul per query tile against a ones vector
  H^T[d, s] = sum_t x[t, d] expP[t, s]      (attn contracts x first)
  out[s, j] = (sum_d H^T[d, s] Wv[d, j]) / l[s] + bv[j]
The Q/K fusion drops the per-core matmul count to 776 N=512 instructions
(13.0 GFLOP/core, below the 15.05 unfused zero-duplication floor) with no
inter-core communication. All matmul operands are bf16 (fp32 PSUM
accumulate): measured relative error ~3e-3. bf16 also halves DMA/SBUF
traffic, so the fused weight, Wv and both x layouts are SBUF-resident for
the whole kernel — every input byte is pulled from HBM exactly once.
"""

import os
import sys
import types

import numpy as np

B, S, D = 4, 2048, 1024
HALF = S // 2  # 1024 queries per core
SCALE = 1.0 / 32.0  # 1/sqrt(D)
NC = 8
DC = D // 128  # 8 d-chunks
TT = S // 128  # 16 key tiles
SBLK = 512  # queries per s-block
NSB = HALF // SBLK  # 2 s-blocks

_CACHED_NC = None
LAST_RESULT = None  # BassKernelResults of the most recent run (for test.py)


def _ensure_axon_ntff_hook():
    """bass_utils' trace path needs antenv.axon_hooks; this image's antenv
    lacks it. Install a shim backed by trn_agent_boot's ctypes hook so
    BASS_TRACE=1 profiling works. No-op if already present/unavailable."""
    try:
        import antenv.axon_hooks  # noqa: F401

        return
    except ImportError:
        pass
    try:
        from trn_agent_boot.trn_boot import _ntff_profile_via_ctypes

        hook = _ntff_profile_via_ctypes("/opt/axon/libaxon_pjrt.so")
    except Exception:
        hook = None
    mod = types.ModuleType("antenv.axon_hooks")
    mod.get_axon_ntff_profile_hook = lambda: hook
    mod.set_axon_ntff_profile_hook = lambda h: None
    sys.modules["antenv.axon_hooks"] = mod


def build_kernel(tc, xt, xn, wkq, wv, ckq, bv, out):
    """Per-core attention with Q/K fused into one projection (see module
    docstring). All matmuls bf16 x bf16 -> fp32 PSUM. Every input is loaded
    exactly once into SBUF up front."""
    import concourse.bass as bass
    from concourse import mybir

    nc = tc.nc
    F32 = mybir.dt.float32
    F32R = mybir.dt.float32r
    BF16 = mybir.dt.bfloat16
    Identity = mybir.ActivationFunctionType.Identity
    Copy = mybir.ActivationFunctionType.Copy
    Exp = mybir.ActivationFunctionType.Exp

    xn_r = xn.rearrange("(tc p) d -> p tc d", p=128)  # [128, 16, 1024]
    out_r = out.rearrange("(su p) j -> su p j", p=128)  # [8, 128, 1024]

    with tc.tile_pool(name="persist", bufs=1) as persist:
        # -- persistent SBUF residents (per-partition bytes in parens) ------
        xT = persist.tile([128, DC, S], BF16)  # 32 KiB
        wkq_sb = persist.tile([128, DC, D], BF16)  # 16 KiB
        G = persist.tile([128, DC, HALF], BF16)  # 16 KiB
        ckq_sb = persist.tile([128, DC], F32)
        ones_f = persist.tile([128, 2], F32)
        ones_t = persist.tile([128, 2], F32R)

        nc.vector.memset(ones_f, 1.0)
        nc.vector.tensor_copy(ones_t, ones_f)

        # All issue queues fan out to the same 16 HW DMA engines (aggregate
        # ~358 GB/s), so the critical prefix (wkq + xT) owns ALL of them:
        # it is issued alone, in first-use order, on the scalar ring. The
        # Phase-B-only tensors (xn, wv, bv) are issued on the idle sync ring
        # from inside the Phase B pool, whose open-barrier naturally delays
        # them until Phase A's matmuls are done competing for HBM.
        # (a single ring tops out near ~200 GB/s; two reach the HBM limit)
        nc.scalar.dma_start(wkq_sb[:, 0:4], wkq[:, 0:4])
        nc.sync.dma_start(wkq_sb[:, 4:8], wkq[:, 4:8])
        nc.scalar.dma_start(xT[:, :, 0:512], xt[0])
        nc.sync.dma_start(xT[:, :, 512:1024], xt[1])
        nc.scalar.dma_start(xT[:, :, 1024:1536], xt[2])
        nc.sync.dma_start(xT[:, :, 1536:2048], xt[3])
        nc.scalar.dma_start(ckq_sb, ckq)

        # ---- Phase A: G = Mkq @ x^T + ckq --------------------------------
        with tc.tile_pool(name="psa", bufs=2, space="PSUM") as psa:
            # PE warmup: tiny input-independent matmuls run during the input
            # DMA wait so the HAM clock gate is at 2.4 GHz when real work
            # arrives (it otherwise starts cold at 1.2 GHz).
            warm = psa.tile([2, 2], F32, tag="warm", bufs=1)
            for _ in range(80):
                nc.tensor.matmul(warm, ones_t, ones_t, start=True, stop=True)

            for sblk in range(NSB):
                for gc in range(DC):
                    gpsum = psa.tile([128, SBLK], F32, tag="gpsum")
                    for c in range(DC):
                        nc.tensor.matmul(
                            gpsum,
                            wkq_sb[:, c, gc * 128 : (gc + 1) * 128],
                            xT[:, c, sblk * SBLK : (sblk + 1) * SBLK],
                            start=(c == 0),
                            stop=(c == DC - 1),
                        )
                    nc.scalar.activation(
                        G[:, gc, sblk * SBLK : (sblk + 1) * SBLK],
                        gpsum,
                        Identity,
                        bias=ckq_sb[:, gc : gc + 1],
                    )

        # ---- Phase B: scores^T -> exp -> H^T -> out, per 512-query block -
        with (
            tc.tile_pool(name="pb", bufs=1) as pb,
            tc.tile_pool(name="psb", bufs=2, space="PSUM") as psb,
        ):
            xn_sb = pb.tile([128, TT, D], BF16, tag="xn")  # 32 KiB
            wv_sb = pb.tile([128, DC, D], BF16, tag="wv")  # 16 KiB
            bv_bc = pb.tile([128, D], F32, tag="bv")  # 4 KiB
            nc.sync.dma_start(xn_sb, xn_r)
            nc.sync.dma_start(wv_sb, wv)
            bv_bcast_ap = bass.AP(
                tensor=bv.tensor, offset=bv.offset, ap=[[0, 128]] + list(bv.ap)
            )
            nc.sync.dma_start(bv_bc, bv_bcast_ap)
            for sb in range(NSB):
                # scores^T + exp; E accumulates the softmax sums on DVE
                expP = pb.tile([128, TT, SBLK], BF16, tag="expP")
                E_t = pb.tile([128, SBLK], F32R, tag="E_t", bufs=1)
                for tt in range(TT):
                    spsum = psb.tile([128, SBLK], F32, tag="spsum")
                    for c in range(DC):
                        nc.tensor.matmul(
                            spsum,
                            xT[:, c, tt * 128 : (tt + 1) * 128],
                            G[:, c, sb * SBLK : (sb + 1) * SBLK],
                            start=(c == 0),
                            stop=(c == DC - 1),
                        )
                    nc.scalar.activation(expP[:, tt, :], spsum, Exp, scale=SCALE)
                    if tt == 1:
                        nc.vector.tensor_add(E_t, expP[:, 0, :], expP[:, 1, :])
                    elif tt > 1:
                        nc.vector.tensor_add(E_t, E_t, expP[:, tt, :])

                # H^T[d, s] = sum_t x[t, d] expP[t, s]
                H = pb.tile([128, DC, SBLK], BF16, tag="H")
                for dc in range(DC):
                    hpsum = psb.tile([128, SBLK], F32, tag="hpsum")
                    for tt in range(TT):
                        nc.tensor.matmul(
                            hpsum,
                            xn_sb[:, tt, dc * 128 : (dc + 1) * 128],
                            expP[:, tt, :],
                            start=(tt == 0),
                            stop=(tt == TT - 1),
                        )
                    nc.scalar.activation(H[:, dc, :], hpsum, Copy)

                # out[s, j] = (sum_d H^T[d, s] Wv[d, j]) / l[s] + bv[j]
                for su in range(SBLK // 128):
                    s0 = su * 128
                    lpsum = psb.tile([128, 2], F32, tag="lpsum", bufs=2)
                    nc.tensor.matmul(
                        lpsum, E_t[:, s0 : s0 + 128], ones_t, start=True, stop=True
                    )
                    recip = pb.tile([128, 1], F32, tag="recip", bufs=2)
                    nc.vector.reciprocal(recip, lpsum[:, 0:1])
                    o_sb = pb.tile([128, D], F32, tag="o_sb", bufs=2)
                    for jb in range(2):
                        opsum = psb.tile([128, 512], F32, tag="opsum")
                        for dc in range(DC):
                            nc.tensor.matmul(
                                opsum,
                                H[:, dc, s0 : s0 + 128],
                                wv_sb[:, dc, jb * 512 : (jb + 1) * 512],
                                start=(dc == 0),
                                stop=(dc == DC - 1),
                            )
                        oh = o_sb[:, jb * 512 : (jb + 1) * 512]
                        nc.vector.tensor_scalar_mul(oh, in0=opsum, scalar1=recip)
                        nc.vector.tensor_add(
                            oh, oh, bv_bc[:, jb * 512 : (jb + 1) * 512]
                        )
                    # one 512 KiB row-block per DMA on the idle sync HW ring
                    # (the gpsimd SWDGE ring is slow to drain the last block)
                    nc.sync.dma_start(out_r[sb * (SBLK // 128) + su], o_sb)


def build_nc():
    global _CACHED_NC
    if _CACHED_NC is not None:
        return _CACHED_NC
    import concourse.tile as tile
    from concourse import bacc, mybir

    F32 = mybir.dt.float32
    BF16 = mybir.dt.bfloat16
    nc = bacc.Bacc("TRN2", target_bir_lowering=False, debug=False)
    xt = nc.dram_tensor(
        "xt", [S // 512, 128, DC, 512], BF16, kind="ExternalInput"
    ).ap()
    xn = nc.dram_tensor("xn", [S, D], BF16, kind="ExternalInput").ap()
    wkq = nc.dram_tensor("wkq", [128, DC, D], BF16, kind="ExternalInput").ap()
    wv = nc.dram_tensor("wv", [128, DC, D], BF16, kind="ExternalInput").ap()
    ckq = nc.dram_tensor("ckq", [128, DC], F32, kind="ExternalInput").ap()
    bv = nc.dram_tensor("bv", [D], F32, kind="ExternalInput").ap()
    out = nc.dram_tensor("out", [HALF, D], F32, kind="ExternalOutput").ap()

    with tile.TileContext(nc) as tc:
        build_kernel(tc, xt, xn, wkq, wv, ckq, bv, out)
    nc.compile()
    _CACHED_NC = nc
    return nc


def _shard_inputs(x, Wq, bq, Wk, bk, Wv, bv):
    """Host-side prep: fused Q/K weight + per-core permuted x layouts."""
    import ml_dtypes

    BF = ml_dtypes.bfloat16
    # scores = q . k = x_t . (Mkq q_s + ckq): Mkq = Wk Wq^T, ckq = Wk bq.
    # The kernel consumes the weight in [d_in, d_out] = Mkq^T layout.
    Mkq = (Wk.astype(np.float64) @ Wq.astype(np.float64).T).astype(np.float32)
    ckq = (Wk.astype(np.float64) @ bq.astype(np.float64)).astype(np.float32)
    wkq_r = np.ascontiguousarray(
        Mkq.T.reshape(DC, 128, D).transpose(1, 0, 2).astype(BF)
    )
    wv_r = np.ascontiguousarray(Wv.reshape(DC, 128, D).transpose(1, 0, 2).astype(BF))
    ckq_r = np.ascontiguousarray(ckq.reshape(DC, 128).T)
    bv_c = np.ascontiguousarray(bv)

    in_maps = []
    for c in range(NC):
        b, h = divmod(c, 2)
        xb = x[b]
        if h:
            xb = np.concatenate([xb[HALF:], xb[:HALF]], axis=0)
        xb16 = xb.astype(BF)
        # x^T pre-tiled as [tb, p, c, u]: xt[tb, p, c, u] = x^T[c*128+p, tb*512+u]
        xt = np.ascontiguousarray(
            xb16.T.reshape(DC, 128, S // 512, 512).transpose(2, 1, 0, 3)
        )
        xn = np.ascontiguousarray(xb16)  # [S, D], same permutation
        in_maps.append(
            {
                "xt": xt,
                "xn": xn,
                "wkq": wkq_r,
                "wv": wv_r,
                "ckq": ckq_r,
                "bv": bv_c,
            }
        )
    return in_maps


def kernel(x, Wq, bq, Wk, bk, Wv, bv):
    global LAST_RESULT
    _ensure_axon_ntff_hook()
    from concourse import bass_utils

    x = np.asarray(x, dtype=np.float32)
    args = [np.asarray(a, dtype=np.float32) for a in (Wq, bq, Wk, bk, Wv, bv)]
    nc = build_nc()
    in_maps = _shard_inputs(x, *args)
    res = bass_utils.run_bass_kernel_spmd(nc, in_maps, core_ids=list(range(NC)))
    LAST_RESULT = res
    out = np.empty((B, S, D), dtype=np.float32)
    for c in range(NC):
        b, h = divmod(c, 2)
        out[b, h * HALF : (h + 1) * HALF, :] = res.results[c]["out"]
    return out


if __name__ == "__main__":
    rng = np.random.default_rng(0)
    init = 1.0 / 32.0
    x = rng.standard_normal((B, S, D), dtype=np.float32)
    mk = lambda *s: rng.uniform(-init, init, s).astype(np.float32)
    o = kernel(x, mk(D, D), mk(D), mk(D, D), mk(D), mk(D, D), mk(D))
    print("out", o.shape, o.dtype, float(np.abs(o).max()))


# revision 26
# speedup vs baseline: 1.0617x; 1.0475x over previous
"""Single-head self-attention (B=4, S=2048, D=1024) on 8 trn2 NeuronCores.

Sharding: core c -> (batch b = c//2, query half h = c%2); data-parallel over
batch, sequence-parallel over queries within a batch. Each core receives its
batch's x in both layouts (x^T d-major for the score projections, x native
t-major for the attention-weighted contraction) with its own seq-half first
(softmax is invariant to key permutation). The host gather is then a pure
concatenation of [1024, 1024] output blocks.

Per-core algorithm (no Q, K^T or V is ever materialized):
  G[d, s] = sum_d' Mkq[d, d'] xT[d', s] + ckq[d]     [1024, 1024]
            where Mkq = Wk @ Wq^T and ckq = Wk @ bq are fused on the host:
            scores[s, t] = q_s . k_t = x_t . (Mkq q_s + ckq) + const(s),
            and const(s) (the K-bias term) cancels in the softmax.
  scores^T[t, s] = sum_d xT[d, t] G[d, s]
  expP = exp(scores^T / 32); E = sum of expP tiles (DVE chain)
  l[s] via one N=2 matmul per query tile against a ones vector
  H^T[d, s] = sum_t x[t, d] expP[t, s]      (attn contracts x first)
  out[s, j] = (sum_d H^T[d, s] Wv[d, j]) / l[s] + bv[j]
The Q/K fusion drops the per-core matmul count to 776 N=512 instructions
(13.0 GFLOP/core, below the 15.05 unfused zero-duplication floor) with no
inter-core communication. All matmul operands are bf16 (fp32 PSUM
accumulate): measured relative error ~3e-3. bf16 also halves DMA/SBUF
traffic, so the fused weight, Wv and both x layouts are SBUF-resident for
the whole kernel — every input byte is pulled from HBM exactly once.
"""

import os
import sys
import types

import numpy as np

B, S, D = 4, 2048, 1024
HALF = S // 2  # 1024 queries per core
SCALE = 1.0 / 32.0  # 1/sqrt(D)
NC = 8
DC = D // 128  # 8 d-chunks
TT = S // 128  # 16 key tiles
SBLK = 512  # queries per s-block
NSB = HALF // SBLK  # 2 s-blocks

_CACHED_NC = None
LAST_RESULT = None  # BassKernelResults of the most recent run (for test.py)


def _ensure_axon_ntff_hook():
    """bass_utils' trace path needs antenv.axon_hooks; this image's antenv
    lacks it. Install a shim backed by trn_agent_boot's ctypes hook so
    BASS_TRACE=1 profiling works. No-op if already present/unavailable."""
    try:
        import antenv.axon_hooks  # noqa: F401

        return
    except ImportError:
        pass
    try:
        from trn_agent_boot.trn_boot import _ntff_profile_via_ctypes

        hook = _ntff_profile_via_ctypes("/opt/axon/libaxon_pjrt.so")
    except Exception:
        hook = None
    mod = types.ModuleType("antenv.axon_hooks")
    mod.get_axon_ntff_profile_hook = lambda: hook
    mod.set_axon_ntff_profile_hook = lambda h: None
    sys.modules["antenv.axon_hooks"] = mod


def build_kernel(tc, xt, xn, wkq, wv, ckq, bv, out):
    """Per-core attention with Q/K fused into one projection (see module
    docstring). All matmuls bf16 x bf16 -> fp32 PSUM. Every input is loaded
    exactly once into SBUF up front."""
    import concourse.bass as bass
    from concourse import mybir

    nc = tc.nc
    F32 = mybir.dt.float32
    F32R = mybir.dt.float32r
    BF16 = mybir.dt.bfloat16
    Identity = mybir.ActivationFunctionType.Identity
    Copy = mybir.ActivationFunctionType.Copy
    Exp = mybir.ActivationFunctionType.Exp

    xn_r = xn.rearrange("(tc p) d -> p tc d", p=128)  # [128, 16, 1024]
    out_r = out.rearrange("(su p) j -> su p j", p=128)  # [8, 128, 1024]

    with tc.tile_pool(name="persist", bufs=1) as persist:
        # -- persistent SBUF residents (per-partition bytes in parens) ------
        xT = persist.tile([128, DC, S], BF16)  # 32 KiB
        wkq_sb = persist.tile([128, DC, D], BF16)  # 16 KiB
        G = persist.tile([128, DC, HALF], BF16)  # 16 KiB
        ckq_sb = persist.tile([128, DC], F32)
        ones_f = persist.tile([128, 2], F32)
        ones_t = persist.tile([128, 2], F32R)

        nc.vector.memset(ones_f, 1.0)
        nc.vector.tensor_copy(ones_t, ones_f)

        # All issue queues fan out to the same 16 HW DMA engines (aggregate
        # ~358 GB/s), so the critical prefix (wkq + xT) owns ALL of them:
        # it is issued alone, in first-use order, on the scalar ring. The
        # Phase-B-only tensors (xn, wv, bv) are issued on the idle sync ring
        # from inside the Phase B pool, whose open-barrier naturally delays
        # them until Phase A's matmuls are done competing for HBM.
        # Which physical DMA ring each queue lands on is a per-run roulette
        # (boot ~8.7us vs ~12us), so the whole critical prefix rides ONE
        # queue in strict first-use order; the half-split lets the first G
        # accumulation wave start after only 1.5 MB has landed.
        nc.scalar.dma_start(wkq_sb[:, 0:4], wkq[:, 0:4])
        nc.scalar.dma_start(xT[:, 0:4, 0:512], xt[0][:, 0:4])
        nc.scalar.dma_start(wkq_sb[:, 4:8], wkq[:, 4:8])
        nc.scalar.dma_start(xT[:, 4:8, 0:512], xt[0][:, 4:8])
        nc.scalar.dma_start(ckq_sb, ckq)
        nc.scalar.dma_start(xT[:, :, 512:1024], xt[1])
        nc.scalar.dma_start(xT[:, :, 1024:1536], xt[2])
        nc.scalar.dma_start(xT[:, :, 1536:2048], xt[3])

        # ---- Phase A: G = Mkq @ x^T + ckq --------------------------------
        with tc.tile_pool(name="psa", bufs=2, space="PSUM") as psa:
            # PE warmup: tiny input-independent matmuls run during the input
            # DMA wait so the HAM clock gate is at 2.4 GHz when real work
            # arrives (it otherwise starts cold at 1.2 GHz).
            warm = psa.tile([2, 2], F32, tag="warm", bufs=1)
            for _ in range(80):
                nc.tensor.matmul(warm, ones_t, ones_t, start=True, stop=True)

            # First s-block: two-pass accumulation over d-chunk halves so the
            # PE starts on wkq[0:4]+xT[0:4] (1.5 MB) while the second half of
            # the prefix is still in flight. gc=0..6 in pass waves (7 PSUM
            # banks + the warm bank = 8); gc=7 runs as a normal group after.
            NW = DC - 1
            gp0 = [
                psa.tile([128, SBLK], F32, tag=f"gp{g}", bufs=1, name=f"gp{g}")
                for g in range(NW)
            ]
            for gc in range(NW):
                for c in range(4):
                    nc.tensor.matmul(
                        gp0[gc],
                        wkq_sb[:, c, gc * 128 : (gc + 1) * 128],
                        xT[:, c, 0:SBLK],
                        start=(c == 0),
                        stop=False,
                        skip_group_check=True,
                    )
            for gc in range(NW):
                for c in range(4, DC):
                    nc.tensor.matmul(
                        gp0[gc],
                        wkq_sb[:, c, gc * 128 : (gc + 1) * 128],
                        xT[:, c, 0:SBLK],
                        start=False,
                        stop=(c == DC - 1),
                        skip_group_check=True,
                    )
                nc.scalar.activation(
                    G[:, gc, 0:SBLK], gp0[gc], Identity, bias=ckq_sb[:, gc : gc + 1]
                )
            nrest = 0
            for sblk, gcs in ((0, range(NW, DC)), (1, range(DC))):
                for gc in gcs:
                    gpsum = gp0[nrest % NW]
                    nrest += 1
                    for c in range(DC):
                        nc.tensor.matmul(
                            gpsum,
                            wkq_sb[:, c, gc * 128 : (gc + 1) * 128],
                            xT[:, c, sblk * SBLK : (sblk + 1) * SBLK],
                            start=(c == 0),
                            stop=(c == DC - 1),
                        )
                    nc.scalar.activation(
                        G[:, gc, sblk * SBLK : (sblk + 1) * SBLK],
                        gpsum,
                        Identity,
                        bias=ckq_sb[:, gc : gc + 1],
                    )

        # ---- Phase B: scores^T -> exp -> H^T -> out, per 512-query block -
        with (
            tc.tile_pool(name="pb", bufs=1) as pb,
            tc.tile_pool(name="psb", bufs=2, space="PSUM") as psb,
        ):
            xn_sb = pb.tile([128, TT, D], BF16, tag="xn")  # 32 KiB
            wv_sb = pb.tile([128, DC, D], BF16, tag="wv")  # 16 KiB
            bv_bc = pb.tile([128, D], F32, tag="bv")  # 4 KiB
            nc.sync.dma_start(xn_sb, xn_r)
            nc.sync.dma_start(wv_sb, wv)
            bv_bcast_ap = bass.AP(
                tensor=bv.tensor, offset=bv.offset, ap=[[0, 128]] + list(bv.ap)
            )
            nc.sync.dma_start(bv_bc, bv_bcast_ap)
            for sb in range(NSB):
                # scores^T + exp; E accumulates the softmax sums on DVE
                expP = pb.tile([128, TT, SBLK], BF16, tag="expP")
                E_t = pb.tile([128, SBLK], F32R, tag="E_t", bufs=1)
                for tt in range(TT):
                    spsum = psb.tile([128, SBLK], F32, tag="spsum")
                    for c in range(DC):
                        nc.tensor.matmul(
                            spsum,
                            xT[:, c, tt * 128 : (tt + 1) * 128],
                            G[:, c, sb * SBLK : (sb + 1) * SBLK],
                            start=(c == 0),
                            stop=(c == DC - 1),
                        )
                    nc.scalar.activation(expP[:, tt, :], spsum, Exp, scale=SCALE)
                    if tt == 1:
                        nc.vector.tensor_add(E_t, expP[:, 0, :], expP[:, 1, :])
                    elif tt > 1:
                        nc.vector.tensor_add(E_t, E_t, expP[:, tt, :])

                # H^T[d, s] = sum_t x[t, d] expP[t, s]
                H = pb.tile([128, DC, SBLK], BF16, tag="H")
                for dc in range(DC):
                    hpsum = psb.tile([128, SBLK], F32, tag="hpsum")
                    for tt in range(TT):
                        nc.tensor.matmul(
                            hpsum,
                            xn_sb[:, tt, dc * 128 : (dc + 1) * 128],
                            expP[:, tt, :],
                            start=(tt == 0),
                            stop=(tt == TT - 1),
                        )
                    nc.scalar.activation(H[:, dc, :], hpsum, Copy)

                # out[s, j] = (sum_d H^T[d, s] Wv[d, j]) / l[s] + bv[j]
                for su in range(SBLK // 128):
                    s0 = su * 128
                    lpsum = psb.tile([128, 2], F32, tag="lpsum", bufs=2)
                    nc.tensor.matmul(
                        lpsum, E_t[:, s0 : s0 + 128], ones_t, start=True, stop=True
                    )
                    recip = pb.tile([128, 1], F32, tag="recip", bufs=2)
                    nc.vector.reciprocal(recip, lpsum[:, 0:1])
                    o_sb = pb.tile([128, D], F32, tag="o_sb", bufs=2)
                    for jb in range(2):
                        opsum = psb.tile([128, 512], F32, tag="opsum")
                        for dc in range(DC):
                            nc.tensor.matmul(
                                opsum,
                                H[:, dc, s0 : s0 + 128],
                                wv_sb[:, dc, jb * 512 : (jb + 1) * 512],
                                start=(dc == 0),
                                stop=(dc == DC - 1),
                            )
                        oh = o_sb[:, jb * 512 : (jb + 1) * 512]
                        nc.vector.tensor_scalar_mul(oh, in0=opsum, scalar1=recip)
                        nc.vector.tensor_add(
                            oh, oh, bv_bc[:, jb * 512 : (jb + 1) * 512]
                        )
                    # one 512 KiB row-block per DMA on the idle sync HW ring
                    # (the gpsimd SWDGE ring is slow to drain the last block)
                    nc.sync.dma_start(out_r[sb * (SBLK // 128) + su], o_sb)


def build_nc():
    global _CACHED_NC
    if _CACHED_NC is not None:
        return _CACHED_NC
    import concourse.tile as tile
    from concourse import bacc, mybir

    F32 = mybir.dt.float32
    BF16 = mybir.dt.bfloat16
    nc = bacc.Bacc("TRN2", target_bir_lowering=False, debug=False)
    xt = nc.dram_tensor(
        "xt", [S // 512, 128, DC, 512], BF16, kind="ExternalInput"
    ).ap()
    xn = nc.dram_tensor("xn", [S, D], BF16, kind="ExternalInput").ap()
    wkq = nc.dram_tensor("wkq", [128, DC, D], BF16, kind="ExternalInput").ap()
    wv = nc.dram_tensor("wv", [128, DC, D], BF16, kind="ExternalInput").ap()
    ckq = nc.dram_tensor("ckq", [128, DC], F32, kind="ExternalInput").ap()
    bv = nc.dram_tensor("bv", [D], F32, kind="ExternalInput").ap()
    out = nc.dram_tensor("out", [HALF, D], F32, kind="ExternalOutput").ap()

    with tile.TileContext(nc) as tc:
        build_kernel(tc, xt, xn, wkq, wv, ckq, bv, out)
    nc.compile()
    _CACHED_NC = nc
    return nc


def _shard_inputs(x, Wq, bq, Wk, bk, Wv, bv):
    """Host-side prep: fused Q/K weight + per-core permuted x layouts."""
    import ml_dtypes

    BF = ml_dtypes.bfloat16
    # scores = q . k = x_t . (Mkq q_s + ckq): Mkq = Wk Wq^T, ckq = Wk bq.
    # The kernel consumes the weight in [d_in, d_out] = Mkq^T layout.
    Mkq = (Wk.astype(np.float64) @ Wq.astype(np.float64).T).astype(np.float32)
    ckq = (Wk.astype(np.float64) @ bq.astype(np.float64)).astype(np.float32)
    wkq_r = np.ascontiguousarray(
        Mkq.T.reshape(DC, 128, D).transpose(1, 0, 2).astype(BF)
    )
    wv_r = np.ascontiguousarray(Wv.reshape(DC, 128, D).transpose(1, 0, 2).astype(BF))
    ckq_r = np.ascontiguousarray(ckq.reshape(DC, 128).T)
    bv_c = np.ascontiguousarray(bv)

    in_maps = []
    for c in range(NC):
        b, h = divmod(c, 2)
        xb = x[b]
        if h:
            xb = np.concatenate([xb[HALF:], xb[:HALF]], axis=0)
        xb16 = xb.astype(BF)
        # x^T pre-tiled as [tb, p, c, u]: xt[tb, p, c, u] = x^T[c*128+p, tb*512+u]
        xt = np.ascontiguousarray(
            xb16.T.reshape(DC, 128, S // 512, 512).transpose(2, 1, 0, 3)
        )
        xn = np.ascontiguousarray(xb16)  # [S, D], same permutation
        in_maps.append(
            {
                "xt": xt,
                "xn": xn,
                "wkq": wkq_r,
                "wv": wv_r,
                "ckq": ckq_r,
                "bv": bv_c,
            }
        )
    return in_maps


def kernel(x, Wq, bq, Wk, bk, Wv, bv):
    global LAST_RESULT
    _ensure_axon_ntff_hook()
    from concourse import bass_utils

    x = np.asarray(x, dtype=np.float32)
    args = [np.asarray(a, dtype=np.float32) for a in (Wq, bq, Wk, bk, Wv, bv)]
    nc = build_nc()
    in_maps = _shard_inputs(x, *args)
    res = bass_utils.run_bass_kernel_spmd(nc, in_maps, core_ids=list(range(NC)))
    LAST_RESULT = res
    out = np.empty((B, S, D), dtype=np.float32)
    for c in range(NC):
        b, h = divmod(c, 2)
        out[b, h * HALF : (h + 1) * HALF, :] = res.results[c]["out"]
    return out


if __name__ == "__main__":
    rng = np.random.default_rng(0)
    init = 1.0 / 32.0
    x = rng.standard_normal((B, S, D), dtype=np.float32)
    mk = lambda *s: rng.uniform(-init, init, s).astype(np.float32)
    o = kernel(x, mk(D, D), mk(D), mk(D, D), mk(D), mk(D, D), mk(D))
    print("out", o.shape, o.dtype, float(np.abs(o).max()))
